# revision 20
# baseline (speedup 1.0000x reference)
"""Trainium2 Bass kernel for DepthAwareGATv2Backbone (2-layer GATConv + JK).

Sharding: nodes partitioned across 8 cores; edges partitioned by destination
node (segment softmax/scatter local); halo exchange of projected source-node
features via on-device AllGather per layer.

Self-contained: hardcodes the problem shapes (N=50000, E=600000, F=128, H=8,
C=16).  kernel(**inputs) takes full unsharded inputs, returns full output.
"""
import math
import sys

import numpy as np
import ml_dtypes

sys.path.insert(0, "/opt/trn_rl_repo")

import concourse.bass as bass  # noqa: E402
import concourse.bacc as bacc  # noqa: E402
import concourse.mybir as mybir  # noqa: E402
import concourse.tile as tile  # noqa: E402
from concourse import bass_utils  # noqa: E402
from concourse.masks import make_identity  # noqa: E402

F32 = mybir.dt.float32
BF = mybir.dt.bfloat16
I32 = mybir.dt.int32
P = 128
LRELU = 0.2
LN_EPS = 1e-5


class Cfg:
    def __init__(self, N, E, NC=8, H=8, C=16, T=None):
        self.N, self.E, self.NC, self.H, self.C = N, E, NC, H, C
        self.D = H * C            # 128
        self.F = 128              # input features
        self.NLOC = N // NC
        self.NBLK = (self.NLOC + P - 1) // P
        self.NLOC_PAD = self.NBLK * P
        self.T = T                # tiles per block (set from data)
        self.TB = None            # per-block tile counts (set from data)

    @property
    def NE_T(self):
        return sum(self.TB) if self.TB is not None else self.NBLK * self.T

    @property
    def EPAD(self):
        return self.NE_T * P


# ---------------------------------------------------------------- host prep

def host_prep(cfg, inputs):
    """Shard/sort/pad edges, build packed weight tensors. Returns in_maps."""
    x = np.asarray(inputs["x"], np.float32)
    ei = np.asarray(inputs["edge_index"])
    src = ei[0].astype(np.int64)
    dst = ei[1].astype(np.int64)
    N, E, NC, H, C, D = cfg.N, cfg.E, cfg.NC, cfg.H, cfg.C, cfg.D

    nsq = (x * x).sum(1, dtype=np.float32)
    xaug = np.zeros((N, 132), np.float32)
    xaug[:, :128] = x
    xaug[:, 128] = nsq

    # --- per-core edge partitions, sorted by dst, grouped into node blocks
    core_of = dst // cfg.NLOC
    T_need = 0
    percore = []
    for c in range(NC):
        m = core_of == c
        s_c, d_c = src[m], dst[m]
        rel = d_c - c * cfg.NLOC
        order = np.argsort(rel, kind="stable")
        s_c, d_c, rel = s_c[order], d_c[order], rel[order]
        blk = rel // P
        counts = np.bincount(blk, minlength=cfg.NBLK)
        T_need = max(T_need, int(np.ceil(counts.max() / P)))
        percore.append((s_c, d_c, rel, blk, counts))
    if cfg.T is None:
        cfg.T = T_need
    assert cfg.T >= T_need, (cfg.T, T_need)
    TB = [max(int(np.ceil(pc[4][b] / P)) for pc in percore)
          for b in range(cfg.NBLK)]
    TB = [max(tb, 1) for tb in TB]
    if sum(TB) % 2:
        TB[-1] += 1
    cfg.TB = TB
    OFF = np.concatenate([[0], np.cumsum(TB)]).astype(int)

    NE_T, EPAD = cfg.NE_T, cfg.EPAD
    in_maps = []

    # --- packed weights (shared across cores)
    in_w = np.asarray(inputs["in_w"], np.float32)
    lin_w1 = np.asarray(inputs["lin_w1"], np.float32)
    lin_w2 = np.asarray(inputs["lin_w2"], np.float32)
    jk_w = np.asarray(inputs["jk_w"], np.float32)

    def fold_att(lin_w, att):
        # W[f, h] = sum_c lin_w[(h,c), f] * att[h, c]
        return np.einsum("hcf,hc->fh", lin_w.reshape(H, C, -1),
                         np.asarray(att, np.float32))

    wa1 = np.concatenate([fold_att(lin_w1, inputs["att_src1"]),
                          fold_att(lin_w1, inputs["att_dst1"])], 1)  # [128,16]
    wa2 = np.concatenate([fold_att(lin_w2, inputs["att_src2"]),
                          fold_att(lin_w2, inputs["att_dst2"])], 1)  # [128,16]
    wmat = np.concatenate(
        [in_w.T, lin_w1.T, lin_w2.T,
         jk_w[:, 0:D].T, jk_w[:, D:2 * D].T, jk_w[:, 2 * D:3 * D].T,
         wa1, wa2], axis=1).astype(np.float32)          # [128, 800]
    assert wmat.shape == (128, 6 * D + 32)
    # bf16 [lin_w1.T | W_asrc1] for the per-edge source projection in phase B
    wmat1b = np.concatenate([lin_w1.T, wa1[:, 0:8]], 1).astype(ml_dtypes.bfloat16)

    brow = np.concatenate(
        [inputs["bias1"], inputs["bias2"], inputs["g1"], inputs["bn1"],
         inputs["g2"], inputs["bn2"], inputs["jk_b"],
         np.asarray(inputs["att_src1"]).reshape(-1),
         np.asarray(inputs["att_dst1"]).reshape(-1),
         np.asarray(inputs["att_src2"]).reshape(-1),
         np.asarray(inputs["att_dst2"]).reshape(-1),
         inputs["in_b"]]).astype(np.float32)[None, :]   # [1, 12*128]

    enc_w1 = np.asarray(inputs["enc_w1"], np.float32)   # [16, 4]
    enc_b1 = np.asarray(inputs["enc_b1"], np.float32)   # [16]
    enc_w2 = np.asarray(inputs["enc_w2"], np.float32)   # [8, 16]
    enc_b2 = np.asarray(inputs["enc_b2"], np.float32)   # [8]

    def mk_M(lin_edge_w, att_edge):
        # a_edge = ea @ M.T ;  M[h, j] = sum_c w[(h,c), j] * att_edge[h, c]
        w3 = lin_edge_w.reshape(H, C, H)
        return np.einsum("hcj,hc->hj", w3, np.asarray(att_edge, np.float32))

    M1 = mk_M(np.asarray(inputs["lin_edge_w1"], np.float32), inputs["att_edge1"])
    M2 = mk_M(np.asarray(inputs["lin_edge_w2"], np.float32), inputs["att_edge2"])
    Mst = np.concatenate([M1, M2], axis=0)              # [16, 8]
    W2m = Mst @ enc_w2                                  # [16, 16]
    b2m = (Mst @ enc_b2).astype(np.float32)             # [16]

    # ea of the (0,0) pad edge, for the pad-correction of sum(ea)
    n0 = nsq[0]
    ef00 = np.array([0.0, n0, n0, 0.0], np.float32)
    ea00 = enc_w2 @ np.maximum(enc_w1 @ ef00 + enc_b1, 0) + enc_b2  # [8]

    for c in range(NC):
        s_c, d_c, rel, blk, counts = percore[c]
        srcrow = np.zeros((NE_T, P), np.int32)
        srcx = np.zeros((NE_T, P), np.int32)
        dstrel = np.full((NE_T, P), 1e9, np.float32)
        pos = np.concatenate([[0], np.cumsum(counts)])
        for b in range(cfg.NBLK):
            cnt = counts[b]
            sl = slice(pos[b], pos[b + 1])
            r0 = OFF[b] * P
            srcx.reshape(-1)[r0:r0 + cnt] = s_c[sl]
            srcrow.reshape(-1)[r0:r0 + cnt] = (
                (s_c[sl] // cfg.NLOC) * cfg.NLOC_PAD + s_c[sl] % cfg.NLOC)
            dstrel.reshape(-1)[r0:r0 + cnt] = rel[sl] - b * P

        def slab(a):
            # [NE_T, P] -> [P, NE_T]: tile t at col t, edge j at partition j
            return np.ascontiguousarray(a.T)

        n_pad = EPAD - len(s_c)
        corr2 = (len(s_c) * enc_b2 - n_pad * (ea00 - enc_b2)).astype(np.float32)

        xt = np.zeros((P, cfg.NLOC_PAD), np.float32)
        xt[:, :cfg.NLOC] = x[c * cfg.NLOC:(c + 1) * cfg.NLOC].T
        xnm = np.zeros((cfg.NBLK, P, 132), np.float32)
        xl = xaug[c * cfg.NLOC:(c + 1) * cfg.NLOC]
        xnm.reshape(cfg.NLOC_PAD, 132)[:cfg.NLOC] = xl
        xnm = np.ascontiguousarray(xnm.transpose(1, 0, 2).reshape(P, cfg.NBLK * 132))

        in_maps.append({
            "xaug": xaug.astype(ml_dtypes.bfloat16),
            "xT": xt,
            "srcrow": slab(srcrow),
            "srcx": slab(srcx),
            "xnm": xnm.astype(ml_dtypes.bfloat16),
            "dstrel": slab(dstrel).astype(ml_dtypes.bfloat16),
            "wmat": wmat,
            "wmat1b": wmat1b,
            "brow": brow,
            "encw1T": np.ascontiguousarray(enc_w1.T).astype(ml_dtypes.bfloat16),
            "encb1": enc_b1[:, None],                          # [16, 1]
            "w2mT": np.ascontiguousarray(W2m.T).astype(ml_dtypes.bfloat16),
            "b2m": b2m[:, None],                               # [16, 1]
            "encw2T": np.ascontiguousarray(enc_w2.T),          # [16, 8]
            "mstT": np.ascontiguousarray(Mst.T),               # [8, 16]
            "corr2": corr2[:, None],                           # [8, 1]
        })
    return in_maps


# ---------------------------------------------------------------- program

def build_program(cfg):
    NC, H, C, D, T, NBLK = cfg.NC, cfg.H, cfg.C, cfg.D, cfg.T, cfg.NBLK
    TB = cfg.TB if cfg.TB is not None else [T] * NBLK
    OFF = [0]
    for tb in TB:
        OFF.append(OFF[-1] + tb)
    BLK_OF = [b for b in range(NBLK) for _ in range(TB[b])]
    NLP = cfg.NLOC_PAD
    NE_T = cfg.NE_T
    HD = H  # 8

    nc = bacc.Bacc("TRN2", target_bir_lowering=False, debug=False,
                   num_devices=NC)

    xaug_d = nc.dram_tensor("xaug", [cfg.N, 132], BF, kind="ExternalInput")
    xT_d = nc.dram_tensor("xT", [P, NLP], F32, kind="ExternalInput")
    srcrow_d = nc.dram_tensor("srcrow", [P, NE_T], I32, kind="ExternalInput")
    srcx_d = nc.dram_tensor("srcx", [P, NE_T], I32, kind="ExternalInput")
    xnm_d = nc.dram_tensor("xnm", [P, NBLK * 132], BF, kind="ExternalInput")
    dstrel_d = nc.dram_tensor("dstrel", [P, NE_T], BF, kind="ExternalInput")
    wmat_d = nc.dram_tensor("wmat", [P, 6 * D + 32], F32, kind="ExternalInput")
    wmat1b_d = nc.dram_tensor("wmat1b", [P, 136], BF, kind="ExternalInput")
    brow_d = nc.dram_tensor("brow", [1, 12 * P], F32, kind="ExternalInput")
    encw1T_d = nc.dram_tensor("encw1T", [4, 16], BF, kind="ExternalInput")
    encb1_d = nc.dram_tensor("encb1", [16, 1], F32, kind="ExternalInput")
    w2mT_d = nc.dram_tensor("w2mT", [16, 16], BF, kind="ExternalInput")
    b2m_d = nc.dram_tensor("b2m", [16, 1], F32, kind="ExternalInput")
    encw2T_d = nc.dram_tensor("encw2T", [16, 8], F32, kind="ExternalInput")
    mstT_d = nc.dram_tensor("mstT", [8, 16], F32, kind="ExternalInput")
    corr2_d = nc.dram_tensor("corr2", [8, 1], F32, kind="ExternalInput")

    out_d = nc.dram_tensor("out", [NLP, D], F32, kind="ExternalOutput")

    ae12_d = nc.dram_tensor("ae12", [cfg.EPAD, 16], BF)
    msg1_d = nc.dram_tensor("msg1", [cfg.EPAD, 136], BF)
    cc2_in = nc.dram_tensor("cc2_in", [NLP, 136], BF)
    cc2_out = nc.dram_tensor("cc2_out", [NC * NLP, 136], BF, addr_space="Shared")
    ccR_in = nc.dram_tensor("ccR_in", [8, 1], F32)
    ccR_out = nc.dram_tensor("ccR_out", [8, 1], F32, addr_space="Shared")
    scr16 = nc.dram_tensor("scr16", [1, 16], F32)

    rg = [list(range(NC))]

    with tile.TileContext(nc) as tc, nc.allow_low_precision(reason="bf16 edge pipeline"):
        with (
            tc.tile_pool(name="slab", bufs=1) as slab,
            tc.tile_pool(name="work", bufs=5) as work,
            tc.tile_pool(name="ps", bufs=2, space="PSUM") as ps,
            tc.tile_pool(name="psacc", bufs=2, space="PSUM") as psacc,
        ):
            # ------------- static tiles
            xT_s = slab.tile([P, NLP], F32, tag="xT")
            srcrow_s = slab.tile([P, NE_T], I32, tag="srcrow")
            srcx_s = slab.tile([P, NE_T], I32, tag="srcx")
            xnm_s = slab.tile([P, NBLK * 132], BF, tag="xnm")
            dstrel_s = slab.tile([P, NE_T], BF, tag="dstrel")
            wmat_s = slab.tile([P, 6 * D + 32], F32, tag="wmat")
            wmat1b_s = slab.tile([P, 136], BF, tag="wmat1b")
            nc.sync.dma_start(out=wmat1b_s[:], in_=wmat1b_d.ap())
            nc.sync.dma_start(out=xT_s[:], in_=xT_d.ap())
            nc.sync.dma_start(out=srcrow_s[:], in_=srcrow_d.ap())
            nc.sync.dma_start(out=srcx_s[:], in_=srcx_d.ap())
            nc.sync.dma_start(out=xnm_s[:], in_=xnm_d.ap())
            nc.sync.dma_start(out=dstrel_s[:], in_=dstrel_d.ap())
            nc.sync.dma_start(out=wmat_s[:], in_=wmat_d.ap())

            brow_names = ["bias1", "bias2", "g1", "bn1", "g2", "bn2", "jkb",
                          "asr1", "adt1", "asr2", "adt2", "inb"]
            brc = {}
            for i, nm in enumerate(brow_names):
                t = slab.tile([P, P], F32, tag=f"brc_{nm}")
                nc.sync.dma_start(
                    out=t[:], in_=brow_d.ap()[:, i * P:(i + 1) * P].to_broadcast([P, P]))
                brc[nm] = t

            encw1T_s = slab.tile([4, 16], BF, tag="encw1T")
            encb1_s = slab.tile([16, 1], F32, tag="encb1")
            w2mT_s = slab.tile([16, 16], BF, tag="w2mT")
            b2m_s = slab.tile([16, 1], F32, tag="b2m")
            encw2T_s = slab.tile([16, 8], F32, tag="encw2T")
            mstT_s = slab.tile([8, 16], F32, tag="mstT")
            corr2_s = slab.tile([8, 1], F32, tag="corr2")
            for t, d in [(encw1T_s, encw1T_d), (encb1_s, encb1_d),
                         (w2mT_s, w2mT_d), (b2m_s, b2m_d), (encw2T_s, encw2T_d),
                         (mstT_s, mstT_d), (corr2_s, corr2_d)]:
                nc.sync.dma_start(out=t[:], in_=d.ap())

            ident = slab.tile([P, P], F32, tag="ident")
            make_identity(nc, ident[:])
            identb = slab.tile([P, P], BF, tag="identb")
            nc.vector.tensor_copy(out=identb[:], in_=ident[:])
            iota2_b = slab.tile([P, 2 * P], BF, tag="iota2b")
            iota_f = slab.tile([P, P], F32, tag="iotaf")
            iota_b = slab.tile([P, P], BF, tag="iotab")
            iota_i = slab.tile([P, P], I32, tag="iotai")
            nc.gpsimd.iota(iota_i[:], pattern=[[1, P]], base=0, channel_multiplier=0)
            nc.vector.tensor_copy(out=iota_f[:], in_=iota_i[:])
            nc.vector.tensor_copy(out=iota_b[:], in_=iota_i[:])
            nc.vector.tensor_copy(out=iota2_b[:, 0:P], in_=iota_b[:])
            nc.vector.tensor_copy(out=iota2_b[:, P:2 * P], in_=iota_b[:])
            eps_s = slab.tile([P, 1], F32, tag="eps")
            nc.vector.memset(eps_s[:], LN_EPS)

            xs1_s = slab.tile([P, NLP], BF, tag="xs1")
            xs2_s = slab.tile([P, NLP], BF, tag="xs2")
            h0_s = slab.tile([P, NLP], F32, tag="h0")
            h1_s = slab.tile([P, NLP], F32, tag="h1")
            h1T_s = slab.tile([P, NLP], F32, tag="h1T")
            asad1_s = slab.tile([P, NBLK * 16], BF, tag="asad1")
            asad2_s = slab.tile([P, NBLK * 16], BF, tag="asad2")
            eaacc_s = slab.tile([16, max(NE_T // 2, 1)], F32, tag="eaacc")
            aself_s = slab.tile([P, 16], F32, tag="aself")

            # ------------- phase A: node projections layer 1 (local only)
            for b in range(NBLK):
                nsl = slice(b * P, (b + 1) * P)
                lhs = xT_s[:, nsl]
                p0 = ps.tile([P, P], F32, tag="pp")
                nc.tensor.matmul(p0[:], lhsT=lhs, rhs=wmat_s[:, 0:D],
                                 start=True, stop=True)
                nc.vector.tensor_add(out=h0_s[:, nsl], in0=p0[:], in1=brc["inb"][:])
                p1 = ps.tile([P, P], F32, tag="pp")
                nc.tensor.matmul(p1[:], lhsT=lhs, rhs=wmat_s[:, D:2 * D],
                                 start=True, stop=True)
                nc.vector.tensor_copy(out=xs1_s[:, nsl], in_=p1[:])
                pa = ps.tile([P, 16], F32, tag="pp")
                nc.tensor.matmul(pa[:], lhsT=lhs, rhs=wmat_s[:, 768:784],
                                 start=True, stop=True)
                nc.vector.tensor_copy(out=asad1_s[:, b * 16:(b + 1) * 16],
                                      in_=pa[:])

            # ------------- phase B: edge features + a_edge for both layers
            for tp in range(NE_T // 2):
                t0 = 2 * tp
                Sf2 = work.tile([P, 2 * P], BF, tag="eS2")
                nc.vector.tensor_tensor(
                    out=Sf2[:].rearrange("p (a q) -> p a q", a=2),
                    in0=dstrel_s[:, t0:t0 + 2].rearrange("p (a o) -> p a o", o=1)
                        .to_broadcast([P, 2, P]),
                    in1=iota2_b[:].rearrange("p (a q) -> p a q", a=2),
                    op=mybir.AluOpType.is_equal)
                efTp2 = ps.tile([4, 2 * P], BF, tag="ppb")
                for k in range(2):
                    t = t0 + k
                    b = BLK_OF[t]
                    gs = work.tile([P, 132], BF, tag="fgs")
                    nc.gpsimd.indirect_dma_start(
                        out=gs[:], out_offset=None, in_=xaug_d.ap(),
                        in_offset=bass.IndirectOffsetOnAxis(ap=srcx_s[:, t:t + 1], axis=0))
                    # stage [xs1|asrc1][src] for layer 1 (read back by plain DMA)
                    gsTp = ps.tile([P, P], BF, tag="ppb")
                    nc.tensor.transpose(out=gsTp[:], in_=gs[:, 0:P],
                                        identity=identb[:])
                    gsT = work.tile([P, P], BF, tag="fgsT")
                    nc.vector.tensor_copy(out=gsT[:], in_=gsTp[:])
                    stg = ps.tile([P, 136], F32, tag="pp")
                    nc.tensor.matmul(stg[:], lhsT=gsT[:], rhs=wmat1b_s[:],
                                     start=True, stop=True)
                    stgb = work.tile([P, 136], BF, tag="fstgb")
                    nc.vector.tensor_copy(out=stgb[:], in_=stg[:])
                    nc.sync.dma_start(out=msg1_d.ap()[t * P:(t + 1) * P, :],
                                      in_=stgb[:])
                    SfTp = ps.tile([P, P], BF, tag="ppb2")
                    nc.tensor.transpose(out=SfTp[:], in_=Sf2[:, k * P:(k + 1) * P],
                                        identity=identb[:])
                    SfT = work.tile([P, P], BF, tag="eST")
                    nc.vector.tensor_copy(out=SfT[:], in_=SfTp[:])
                    gdp = ps.tile([P, 132], F32, tag="pp")
                    nc.tensor.matmul(gdp[:], lhsT=SfT[:],
                                     rhs=xnm_s[:, b * 132:(b + 1) * 132],
                                     start=True, stop=True)
                    prod = work.tile([P, P], F32, tag="fprod", bufs=3)
                    nc.vector.tensor_mul(out=prod[:], in0=gs[:, 0:P], in1=gdp[:, 0:P])
                    ef = work.tile([P, 4], BF, tag="fef")
                    nc.vector.reduce_sum(out=ef[:, 0:1], in_=prod[:],
                                         axis=mybir.AxisListType.X)
                    nn = work.tile([P, 2], F32, tag="fnn")
                    nc.vector.tensor_add(out=nn[:, 0:1], in0=gs[:, 128:129],
                                         in1=gdp[:, 128:129])
                    nc.vector.tensor_scalar(out=nn[:, 1:2], in0=ef[:, 0:1],
                                            scalar1=-2.0, scalar2=None,
                                            op0=mybir.AluOpType.mult)
                    nc.vector.tensor_add(out=ef[:, 1:2], in0=nn[:, 0:1], in1=nn[:, 1:2])
                    nc.vector.tensor_copy(out=ef[:, 2:3], in_=gs[:, 128:129])
                    nc.vector.tensor_copy(out=ef[:, 3:4], in_=gdp[:, 128:129])
                    nc.tensor.transpose(out=efTp2[:, k * P:(k + 1) * P], in_=ef[:],
                                        identity=identb[:])
                efT2 = work.tile([4, 2 * P], BF, tag="fefT")
                nc.vector.tensor_copy(out=efT2[:], in_=efTp2[:])
                h1ep = ps.tile([16, 2 * P], F32, tag="pp")
                nc.tensor.matmul(h1ep[:], lhsT=encw1T_s[:], rhs=efT2[:],
                                 start=True, stop=True)
                h1e = work.tile([16, 2 * P], BF, tag="fh1e")
                nc.scalar.activation(out=h1e[:], in_=h1ep[:],
                                     func=mybir.ActivationFunctionType.Relu,
                                     bias=encb1_s[:],
                                     accum_out=eaacc_s[:, tp:tp + 1])
                aep = ps.tile([16, 2 * P], F32, tag="pp")
                nc.tensor.matmul(aep[:], lhsT=w2mT_s[:], rhs=h1e[:],
                                 start=True, stop=True)
                aeT = work.tile([16, 2 * P], BF, tag="faeT")
                nc.scalar.activation(out=aeT[:], in_=aep[:],
                                     func=mybir.ActivationFunctionType.Identity,
                                     bias=b2m_s[:])
                aeRp = ps.tile([P, 32], BF, tag="ppb2")
                for k in range(2):
                    nc.tensor.transpose(out=aeRp[:, k * 16:(k + 1) * 16],
                                        in_=aeT[:, k * P:(k + 1) * P],
                                        identity=identb[:16, :16])
                aeR = work.tile([P, 32], BF, tag="faeR")
                nc.vector.tensor_copy(out=aeR[:], in_=aeRp[:])
                nc.sync.dma_start(
                    out=ae12_d.ap()[t0 * P:(t0 + 2) * P, :]
                        .rearrange("(a p) j -> p a j", p=P),
                    in_=aeR[:].rearrange("p (a j) -> p a j", a=2))

            # sum of ea over real edges -> AllReduce -> a_self for both layers
            sr = work.tile([16, 1], F32, tag="sr")
            nc.vector.reduce_sum(out=sr[:], in_=eaacc_s[:],
                                 axis=mybir.AxisListType.X)
            eap = ps.tile([8, 1], F32, tag="pp")
            nc.tensor.matmul(eap[:], lhsT=encw2T_s[:], rhs=sr[:],
                             start=True, stop=True)
            eas = work.tile([8, 1], F32, tag="eas")
            nc.vector.tensor_add(out=eas[:], in0=eap[:], in1=corr2_s[:])
            nc.sync.dma_start(out=ccR_in.ap(), in_=eas[:])
            nc.gpsimd.collective_compute(
                "AllReduce", mybir.AluOpType.add, replica_groups=rg,
                ins=[ccR_in.ap().opt()], outs=[ccR_out.ap().opt()])
            gsum = work.tile([8, 1], F32, tag="gsum")
            nc.sync.dma_start(out=gsum[:], in_=ccR_out.ap())
            mean8 = work.tile([8, 1], F32, tag="mean8")
            nc.vector.tensor_scalar(out=mean8[:], in0=gsum[:],
                                    scalar1=1.0 / cfg.E, scalar2=None,
                                    op0=mybir.AluOpType.mult)
            asp = ps.tile([16, 1], F32, tag="pp")
            nc.tensor.matmul(asp[:], lhsT=mstT_s[:], rhs=mean8[:],
                             start=True, stop=True)
            c16 = work.tile([16, 1], F32, tag="c16")
            nc.vector.tensor_copy(out=c16[:], in_=asp[:])
            r16p = ps.tile([1, 16], F32, tag="pp")
            nc.tensor.transpose(out=r16p[:], in_=c16[:], identity=ident[:16, :16])
            r16 = work.tile([1, 16], F32, tag="r16")
            nc.vector.tensor_copy(out=r16[:], in_=r16p[:])
            nc.sync.dma_start(out=scr16.ap(), in_=r16[:])
            nc.sync.dma_start(out=aself_s[:], in_=scr16.ap().to_broadcast([P, 16]))

            # ------------- GAT layers
            def gat_layer(lyr, cco, xs_s, asad_s, hres_s, hpre_s):
                """lyr: 0 or 1. hres_s: residual input (h0 or h1).
                hpre_s: where pre-LN output rows go (LN applied in a later
                batched pass). Layer 0 loads staged per-edge messages from
                msg1_d with plain DMAs; layer 1 gathers from cc2_out."""
                ao = lyr * 8
                gdt = BF
                gbias = brc["bias1"] if lyr == 0 else brc["bias2"]
                for b in range(NBLK):
                    nsl = slice(b * P, (b + 1) * P)
                    asl = slice(b * 16, b * 16 + 8)
                    dsl = slice(b * 16 + 8, b * 16 + 16)
                    acc = psacc.tile([P, 136], F32, tag="acc")
                    for tp2 in range(TB[b] // 2 + (TB[b] % 2)):
                        tt0 = 2 * tp2
                        npair = 2 if tt0 + 1 < TB[b] else 1
                        t0 = OFF[b] + tt0
                        gxs = work.tile([P, 2 * 136], gdt, tag="egxs", bufs=8)
                        if lyr == 0:
                            nc.sync.dma_start(
                                out=gxs[:, 0:npair * 136]
                                    .rearrange("p (a j) -> p a j", j=136),
                                in_=msg1_d.ap()[t0 * P:(t0 + npair) * P, :]
                                    .rearrange("(a p) j -> p a j", p=P))
                        else:
                            for k in range(npair):
                                nc.gpsimd.indirect_dma_start(
                                    out=gxs[:, k * 136:(k + 1) * 136],
                                    out_offset=None,
                                    in_=cco.ap(),
                                    in_offset=bass.IndirectOffsetOnAxis(
                                        ap=srcrow_s[:, t0 + k:t0 + k + 1], axis=0))
                        aeR = work.tile([P, 32], BF, tag="eaeR")
                        nc.sync.dma_start(
                            out=aeR[:, 0:npair * 16].rearrange("p (a j) -> p a j", j=16),
                            in_=ae12_d.ap()[t0 * P:(t0 + npair) * P, :]
                                .rearrange("(a p) j -> p a j", p=P))
                        S2 = work.tile([P, 2 * P], BF, tag="gS2")
                        nc.vector.tensor_tensor(
                            out=S2[:, 0:npair * P].rearrange("p (a q) -> p a q", q=P),
                            in0=dstrel_s[:, t0:t0 + npair]
                                .rearrange("p (a o) -> p a o", o=1)
                                .to_broadcast([P, npair, P]),
                            in1=iota2_b[:, 0:npair * P].rearrange("p (a q) -> p a q", q=P),
                            op=mybir.AluOpType.is_equal)
                        adp = ps.tile([P, 16], F32, tag="ppb2")
                        for k in range(npair):
                            STp = ps.tile([P, P], BF, tag="ppb")
                            nc.tensor.transpose(out=STp[:], in_=S2[:, k * P:(k + 1) * P],
                                                identity=identb[:])
                            ST = work.tile([P, P], BF, tag="gST")
                            nc.vector.tensor_copy(out=ST[:], in_=STp[:])
                            nc.tensor.matmul(adp[:, k * 8:(k + 1) * 8], lhsT=ST[:],
                                             rhs=asad_s[:, dsl],
                                             start=True, stop=True)
                        al = work.tile([P, 16], F32, tag="eal")
                        nc.vector.tensor_add(
                            out=al[:, 0:npair * 8].rearrange("p (a q) -> p a q", q=8),
                            in0=gxs[:].rearrange("p (a q) -> p a q", q=136)[
                                :, 0:npair, D:136],
                            in1=adp[:, 0:npair * 8].rearrange("p (a q) -> p a q", q=8))
                        nc.vector.tensor_add(
                            out=al[:, 0:npair * 8].rearrange("p (a q) -> p a q", q=8),
                            in0=al[:, 0:npair * 8].rearrange("p (a q) -> p a q", q=8),
                            in1=aeR[:, 0:npair * 16].rearrange("p (a j) -> p a j", j=16)[
                                :, :, ao:ao + 8])
                        al2 = work.tile([P, 16], F32, tag="eal2")
                        nc.vector.tensor_scalar(out=al2[:, 0:npair * 8],
                                                in0=al[:, 0:npair * 8], scalar1=LRELU,
                                                scalar2=None, op0=mybir.AluOpType.mult)
                        nc.vector.tensor_tensor(out=al[:, 0:npair * 8],
                                                in0=al[:, 0:npair * 8],
                                                in1=al2[:, 0:npair * 8],
                                                op=mybir.AluOpType.max)
                        msg = work.tile([P, 2 * 136], BF, tag="emsg")
                        nc.scalar.activation(
                            out=msg[:].rearrange("p (a q) -> p a q", q=136)[
                                :, 0:npair, D:136],
                            in_=al[:, 0:npair * 8].rearrange("p (a q) -> p a q", q=8),
                            func=mybir.ActivationFunctionType.Exp)
                        nc.vector.tensor_tensor(
                            out=msg[:].rearrange("p (a q) -> p a q", q=136)[
                                :, 0:npair, 0:D].rearrange(
                                "p a (hh cc) -> p a hh cc", hh=H),
                            in0=gxs[:].rearrange("p (a q) -> p a q", q=136)[
                                :, 0:npair, 0:D].rearrange(
                                "p a (hh cc) -> p a hh cc", hh=H),
                            in1=msg[:].rearrange("p (a q) -> p a q", q=136)[
                                :, 0:npair, D:136].rearrange(
                                "p a (hh o) -> p a hh o", o=1)
                                .to_broadcast([P, npair, H, C]),
                            op=mybir.AluOpType.mult)
                        for k in range(npair):
                            nc.tensor.matmul(
                                acc[:], lhsT=S2[:, k * P:(k + 1) * P],
                                rhs=msg[:, k * 136:(k + 1) * 136],
                                start=(tt0 + k == 0), stop=(tt0 + k == TB[b] - 1))
                    # ---- block flush: self loop + normalize + elu + residual
                    als = work.tile([P, 8], F32, tag="fal")
                    nc.vector.tensor_add(out=als[:], in0=asad_s[:, asl],
                                         in1=asad_s[:, dsl])
                    nc.vector.tensor_add(out=als[:], in0=als[:],
                                         in1=aself_s[:, ao:ao + 8])
                    als2 = work.tile([P, 8], F32, tag="fal2")
                    nc.vector.tensor_scalar(out=als2[:], in0=als[:], scalar1=LRELU,
                                            scalar2=None, op0=mybir.AluOpType.mult)
                    nc.vector.tensor_tensor(out=als[:], in0=als[:], in1=als2[:],
                                            op=mybir.AluOpType.max)
                    exps = work.tile([P, 8], F32, tag="fexp")
                    nc.scalar.activation(out=exps[:], in_=als[:],
                                         func=mybir.ActivationFunctionType.Exp)
                    den = work.tile([P, 8], F32, tag="fden")
                    nc.vector.tensor_add(out=den[:], in0=acc[:, D:136], in1=exps[:])
                    nc.vector.reciprocal(out=den[:], in_=den[:])
                    smsg = work.tile([P, P], F32, tag="fsmsg")
                    nc.vector.tensor_tensor(
                        out=smsg[:].rearrange("p (h c) -> p h c", h=H),
                        in0=xs_s[:, nsl].rearrange("p (h c) -> p h c", h=H),
                        in1=exps[:].rearrange("p (h o) -> p h o", o=1)
                            .to_broadcast([P, H, C]),
                        op=mybir.AluOpType.mult)
                    num = work.tile([P, P], F32, tag="fnum")
                    nc.vector.tensor_add(out=num[:], in0=acc[:, 0:D], in1=smsg[:])
                    nc.vector.tensor_tensor(
                        out=num[:].rearrange("p (h c) -> p h c", h=H),
                        in0=num[:].rearrange("p (h c) -> p h c", h=H),
                        in1=den[:].rearrange("p (h o) -> p h o", o=1)
                            .to_broadcast([P, H, C]),
                        op=mybir.AluOpType.mult)
                    nc.vector.tensor_add(out=num[:], in0=num[:], in1=gbias[:])
                    # elu(x) = max(x,0) + (exp(min(x,0)) - 1)
                    mn = work.tile([P, P], F32, tag="fmn")
                    nc.vector.tensor_scalar(out=mn[:], in0=num[:], scalar1=0.0,
                                            scalar2=None, op0=mybir.AluOpType.min)
                    nc.scalar.activation(out=mn[:], in_=mn[:],
                                         func=mybir.ActivationFunctionType.Exp)
                    nc.vector.tensor_scalar(out=num[:], in0=num[:], scalar1=0.0,
                                            scalar2=None, op0=mybir.AluOpType.max)
                    nc.vector.tensor_add(out=num[:], in0=num[:], in1=mn[:])
                    nc.vector.tensor_scalar(out=num[:], in0=num[:], scalar1=-1.0,
                                            scalar2=None, op0=mybir.AluOpType.add)
                    # residual; LN deferred to a batched pass
                    nc.vector.tensor_add(out=hpre_s[:, nsl], in0=num[:],
                                         in1=hres_s[:, nsl])

            def ln_block(src_ap, gg, gb):
                """LayerNorm src_ap in place; returns normalized tile view."""
                st6 = work.tile([P, 6], F32, tag="fst6")
                nc.vector.bn_stats(out=st6[:], in_=src_ap)
                mv = work.tile([P, 2], F32, tag="fmv")
                nc.vector.bn_aggr(out=mv[:], in_=st6[:])
                sd = work.tile([P, 1], F32, tag="fsd")
                nc.scalar.activation(out=sd[:], in_=mv[:, 1:2],
                                     func=mybir.ActivationFunctionType.Sqrt,
                                     bias=eps_s[:])
                nc.vector.reciprocal(out=sd[:], in_=sd[:])
                nc.vector.tensor_scalar(out=src_ap, in0=src_ap,
                                        scalar1=mv[:, 0:1], scalar2=sd[:],
                                        op0=mybir.AluOpType.subtract,
                                        op1=mybir.AluOpType.mult)
                nc.vector.tensor_mul(out=src_ap, in0=src_ap, in1=gg[:])
                nc.vector.tensor_add(out=src_ap, in0=src_ap, in1=gb[:])

            gat_layer(0, None, xs1_s, asad1_s, h0_s, h1_s)

            # ------------- LN pass layer 1 (batched: one act-table swap)
            for b in range(NBLK):
                nsl = slice(b * P, (b + 1) * P)
                ln_block(h1_s[:, nsl], brc["g1"], brc["bn1"])
                h1Tp = ps.tile([P, P], F32, tag="pp")
                nc.tensor.transpose(out=h1Tp[:], in_=h1_s[:, nsl],
                                    identity=ident[:])
                nc.vector.tensor_copy(out=h1T_s[:, nsl], in_=h1Tp[:])

            # ------------- phase D: node projections layer 2
            for b in range(NBLK):
                nsl = slice(b * P, (b + 1) * P)
                p2 = ps.tile([P, P], F32, tag="pp")
                nc.tensor.matmul(p2[:], lhsT=h1T_s[:, nsl], rhs=wmat_s[:, 2 * D:3 * D],
                                 start=True, stop=True)
                nc.vector.tensor_copy(out=xs2_s[:, nsl], in_=p2[:])
                pa2 = ps.tile([P, 16], F32, tag="pp")
                nc.tensor.matmul(pa2[:], lhsT=h1T_s[:, nsl],
                                 rhs=wmat_s[:, 784:800], start=True, stop=True)
                nc.vector.tensor_copy(out=asad2_s[:, b * 16:(b + 1) * 16],
                                      in_=pa2[:])
                nc.sync.dma_start(out=cc2_in.ap()[nsl, 0:D], in_=xs2_s[:, nsl])
                nc.sync.dma_start(out=cc2_in.ap()[nsl, D:136],
                                  in_=asad2_s[:, b * 16:b * 16 + 8])

            nc.gpsimd.collective_compute(
                "AllGather", mybir.AluOpType.bypass, replica_groups=rg,
                ins=[cc2_in.ap().opt()], outs=[cc2_out.ap().opt()])

            # layer 2 writes pre-LN h2 into xT_s (no longer needed)
            gat_layer(1, cc2_out, xs2_s, asad2_s, h1_s, xT_s)

            # ------------- LN2 + JK + output (batched act tables)
            for b in range(NBLK):
                nsl = slice(b * P, (b + 1) * P)
                ln_block(xT_s[:, nsl], brc["g2"], brc["bn2"])
                h2Tp = ps.tile([P, P], F32, tag="pp")
                nc.tensor.transpose(out=h2Tp[:], in_=xT_s[:, nsl],
                                    identity=ident[:])
                h2T = work.tile([P, P], F32, tag="fh2T")
                nc.vector.tensor_copy(out=h2T[:], in_=h2Tp[:])
                h0Tp = ps.tile([P, P], F32, tag="pp")
                nc.tensor.transpose(out=h0Tp[:], in_=h0_s[:, nsl],
                                    identity=ident[:])
                h0T = work.tile([P, P], F32, tag="fh0T")
                nc.vector.tensor_copy(out=h0T[:], in_=h0Tp[:])
                jkp = psacc.tile([P, P], F32, tag="acc")
                nc.tensor.matmul(jkp[:], lhsT=h0T[:],
                                 rhs=wmat_s[:, 3 * D:4 * D],
                                 start=True, stop=False)
                nc.tensor.matmul(jkp[:], lhsT=h1T_s[:, nsl],
                                 rhs=wmat_s[:, 4 * D:5 * D],
                                 start=False, stop=False)
                nc.tensor.matmul(jkp[:], lhsT=h2T[:],
                                 rhs=wmat_s[:, 5 * D:6 * D],
                                 start=False, stop=True)
                outt = work.tile([P, P], F32, tag="foutt")
                nc.vector.tensor_add(out=outt[:], in0=jkp[:], in1=brc["jkb"][:])
                nc.sync.dma_start(out=out_d.ap()[nsl, :], in_=outt[:])

    nc.compile()
    return nc


# ---------------------------------------------------------------- entry

FULL_CFG = dict(N=50000, E=600000, NC=8, H=8, C=16)


def kernel(**inputs):
    cfg = Cfg(**FULL_CFG)
    in_maps = host_prep(cfg, inputs)
    nc = build_program(cfg)
    res = bass_utils.run_bass_kernel_spmd(
        nc, in_maps, core_ids=list(range(cfg.NC)))
    out = np.concatenate(
        [res.results[c]["out"][:cfg.NLOC] for c in range(cfg.NC)], axis=0)
    return out.astype(np.float32)



# revision 27
# speedup vs baseline: 1.0082x; 1.0082x over previous
"""Trainium2 Bass kernel for DepthAwareGATv2Backbone (2-layer GATConv + JK).

Sharding: nodes partitioned across 8 cores; edges partitioned by destination
node (segment softmax/scatter local); halo exchange of projected source-node
features via on-device AllGather per layer.

Self-contained: hardcodes the problem shapes (N=50000, E=600000, F=128, H=8,
C=16).  kernel(**inputs) takes full unsharded inputs, returns full output.
"""
import math
import sys

import numpy as np
import ml_dtypes

sys.path.insert(0, "/opt/trn_rl_repo")

import concourse.bass as bass  # noqa: E402
import concourse.bacc as bacc  # noqa: E402
import concourse.mybir as mybir  # noqa: E402
import concourse.tile as tile  # noqa: E402
from concourse import bass_utils  # noqa: E402
from concourse.masks import make_identity  # noqa: E402

F32 = mybir.dt.float32
BF = mybir.dt.bfloat16
I32 = mybir.dt.int32
P = 128
LRELU = 0.2
LN_EPS = 1e-5


class Cfg:
    def __init__(self, N, E, NC=8, H=8, C=16, T=None):
        self.N, self.E, self.NC, self.H, self.C = N, E, NC, H, C
        self.D = H * C            # 128
        self.F = 128              # input features
        self.NLOC = N // NC
        self.NBLK = (self.NLOC + P - 1) // P
        self.NLOC_PAD = self.NBLK * P
        self.T = T                # tiles per block (set from data)
        self.TB = None            # per-block tile counts (set from data)

    @property
    def NE_T(self):
        return sum(self.TB) if self.TB is not None else self.NBLK * self.T

    @property
    def EPAD(self):
        return self.NE_T * P


# ---------------------------------------------------------------- host prep

def host_prep(cfg, inputs):
    """Shard/sort/pad edges, build packed weight tensors. Returns in_maps."""
    x = np.asarray(inputs["x"], np.float32)
    ei = np.asarray(inputs["edge_index"])
    src = ei[0].astype(np.int64)
    dst = ei[1].astype(np.int64)
    N, E, NC, H, C, D = cfg.N, cfg.E, cfg.NC, cfg.H, cfg.C, cfg.D

    nsq = (x * x).sum(1, dtype=np.float32)
    xaug = np.zeros((N, 132), np.float32)
    xaug[:, :128] = x
    xaug[:, 128] = nsq

    # --- per-core edge partitions, sorted by dst, grouped into node blocks
    core_of = dst // cfg.NLOC
    T_need = 0
    percore = []
    for c in range(NC):
        m = core_of == c
        s_c, d_c = src[m], dst[m]
        rel = d_c - c * cfg.NLOC
        order = np.argsort(rel, kind="stable")
        s_c, d_c, rel = s_c[order], d_c[order], rel[order]
        blk = rel // P
        counts = np.bincount(blk, minlength=cfg.NBLK)
        T_need = max(T_need, int(np.ceil(counts.max() / P)))
        percore.append((s_c, d_c, rel, blk, counts))
    if cfg.T is None:
        cfg.T = T_need
    assert cfg.T >= T_need, (cfg.T, T_need)
    TB = [max(int(np.ceil(pc[4][b] / P)) for pc in percore)
          for b in range(cfg.NBLK)]
    TB = [max(tb, 1) for tb in TB]
    if sum(TB) % 2:
        TB[-1] += 1
    cfg.TB = TB
    OFF = np.concatenate([[0], np.cumsum(TB)]).astype(int)

    NE_T, EPAD = cfg.NE_T, cfg.EPAD
    in_maps = []

    # --- packed weights (shared across cores)
    in_w = np.asarray(inputs["in_w"], np.float32)
    lin_w1 = np.asarray(inputs["lin_w1"], np.float32)
    lin_w2 = np.asarray(inputs["lin_w2"], np.float32)
    jk_w = np.asarray(inputs["jk_w"], np.float32)

    def fold_att(lin_w, att):
        # W[f, h] = sum_c lin_w[(h,c), f] * att[h, c]
        return np.einsum("hcf,hc->fh", lin_w.reshape(H, C, -1),
                         np.asarray(att, np.float32))

    wa1 = np.concatenate([fold_att(lin_w1, inputs["att_src1"]),
                          fold_att(lin_w1, inputs["att_dst1"])], 1)  # [128,16]
    wa2 = np.concatenate([fold_att(lin_w2, inputs["att_src2"]),
                          fold_att(lin_w2, inputs["att_dst2"])], 1)  # [128,16]
    wmat = np.concatenate(
        [in_w.T, lin_w1.T, lin_w2.T,
         jk_w[:, 0:D].T, jk_w[:, D:2 * D].T, jk_w[:, 2 * D:3 * D].T,
         wa1, wa2], axis=1).astype(np.float32)          # [128, 800]
    assert wmat.shape == (128, 6 * D + 32)
    # bf16 [lin_w1.T | W_asrc1] for the per-edge source projection in phase B
    wmat1b = np.concatenate([lin_w1.T, wa1[:, 0:8]], 1).astype(ml_dtypes.bfloat16)

    brow = np.concatenate(
        [inputs["bias1"], inputs["bias2"], inputs["g1"], inputs["bn1"],
         inputs["g2"], inputs["bn2"], inputs["jk_b"],
         np.asarray(inputs["att_src1"]).reshape(-1),
         np.asarray(inputs["att_dst1"]).reshape(-1),
         np.asarray(inputs["att_src2"]).reshape(-1),
         np.asarray(inputs["att_dst2"]).reshape(-1),
         inputs["in_b"]]).astype(np.float32)[None, :]   # [1, 12*128]

    enc_w1 = np.asarray(inputs["enc_w1"], np.float32)   # [16, 4]
    enc_b1 = np.asarray(inputs["enc_b1"], np.float32)   # [16]
    enc_w2 = np.asarray(inputs["enc_w2"], np.float32)   # [8, 16]
    enc_b2 = np.asarray(inputs["enc_b2"], np.float32)   # [8]

    def mk_M(lin_edge_w, att_edge):
        # a_edge = ea @ M.T ;  M[h, j] = sum_c w[(h,c), j] * att_edge[h, c]
        w3 = lin_edge_w.reshape(H, C, H)
        return np.einsum("hcj,hc->hj", w3, np.asarray(att_edge, np.float32))

    M1 = mk_M(np.asarray(inputs["lin_edge_w1"], np.float32), inputs["att_edge1"])
    M2 = mk_M(np.asarray(inputs["lin_edge_w2"], np.float32), inputs["att_edge2"])
    Mst = np.concatenate([M1, M2], axis=0)              # [16, 8]
    W2m = Mst @ enc_w2                                  # [16, 16]
    b2m = (Mst @ enc_b2).astype(np.float32)             # [16]

    # ea of the (0,0) pad edge, for the pad-correction of sum(ea)
    n0 = nsq[0]
    ef00 = np.array([0.0, n0, n0, 0.0], np.float32)
    ea00 = enc_w2 @ np.maximum(enc_w1 @ ef00 + enc_b1, 0) + enc_b2  # [8]

    for c in range(NC):
        s_c, d_c, rel, blk, counts = percore[c]
        srcrow = np.zeros((NE_T, P), np.int32)
        srcx = np.zeros((NE_T, P), np.int32)
        dstrel = np.full((NE_T, P), 1e9, np.float32)
        pos = np.concatenate([[0], np.cumsum(counts)])
        for b in range(cfg.NBLK):
            cnt = counts[b]
            sl = slice(pos[b], pos[b + 1])
            r0 = OFF[b] * P
            srcx.reshape(-1)[r0:r0 + cnt] = s_c[sl]
            srcrow.reshape(-1)[r0:r0 + cnt] = (
                (s_c[sl] // cfg.NLOC) * cfg.NLOC_PAD + s_c[sl] % cfg.NLOC)
            dstrel.reshape(-1)[r0:r0 + cnt] = rel[sl] - b * P

        def slab(a):
            # [NE_T, P] -> [P, NE_T]: tile t at col t, edge j at partition j
            return np.ascontiguousarray(a.T)

        n_pad = EPAD - len(s_c)
        corr2 = (len(s_c) * enc_b2 - n_pad * (ea00 - enc_b2)).astype(np.float32)

        xt = np.zeros((P, cfg.NLOC_PAD), np.float32)
        xt[:, :cfg.NLOC] = x[c * cfg.NLOC:(c + 1) * cfg.NLOC].T
        xnm = np.zeros((cfg.NBLK, P, 132), np.float32)
        xl = xaug[c * cfg.NLOC:(c + 1) * cfg.NLOC]
        xnm.reshape(cfg.NLOC_PAD, 132)[:cfg.NLOC] = xl
        xnm = np.ascontiguousarray(xnm.transpose(1, 0, 2).reshape(P, cfg.NBLK * 132))

        in_maps.append({
            "xaug": xaug.astype(ml_dtypes.bfloat16),
            "xT": xt,
            "srcrow": slab(srcrow),
            "srcx": slab(srcx),
            "xnm": xnm.astype(ml_dtypes.bfloat16),
            "dstrel": slab(dstrel).astype(ml_dtypes.bfloat16),
            "dstrelT": dstrel.reshape(1, -1).astype(ml_dtypes.bfloat16),
            "wmat": wmat,
            "wmat1b": wmat1b,
            "brow": brow,
            "encw1T": np.ascontiguousarray(enc_w1.T).astype(ml_dtypes.bfloat16),
            "encb1": enc_b1[:, None],                          # [16, 1]
            "w2mT": np.ascontiguousarray(W2m.T).astype(ml_dtypes.bfloat16),
            "b2m": b2m[:, None],                               # [16, 1]
            "encw2T": np.ascontiguousarray(enc_w2.T),          # [16, 8]
            "mstT": np.ascontiguousarray(Mst.T),               # [8, 16]
            "corr2": corr2[:, None],                           # [8, 1]
        })
    return in_maps


# ---------------------------------------------------------------- program

def build_program(cfg):
    NC, H, C, D, T, NBLK = cfg.NC, cfg.H, cfg.C, cfg.D, cfg.T, cfg.NBLK
    TB = cfg.TB if cfg.TB is not None else [T] * NBLK
    OFF = [0]
    for tb in TB:
        OFF.append(OFF[-1] + tb)
    BLK_OF = [b for b in range(NBLK) for _ in range(TB[b])]
    NLP = cfg.NLOC_PAD
    NE_T = cfg.NE_T
    HD = H  # 8

    nc = bacc.Bacc("TRN2", target_bir_lowering=False, debug=False,
                   num_devices=NC)

    xaug_d = nc.dram_tensor("xaug", [cfg.N, 132], BF, kind="ExternalInput")
    xT_d = nc.dram_tensor("xT", [P, NLP], F32, kind="ExternalInput")
    srcrow_d = nc.dram_tensor("srcrow", [P, NE_T], I32, kind="ExternalInput")
    srcx_d = nc.dram_tensor("srcx", [P, NE_T], I32, kind="ExternalInput")
    xnm_d = nc.dram_tensor("xnm", [P, NBLK * 132], BF, kind="ExternalInput")
    dstrel_d = nc.dram_tensor("dstrel", [P, NE_T], BF, kind="ExternalInput")
    dstrelT_d = nc.dram_tensor("dstrelT", [1, NE_T * P], BF, kind="ExternalInput")
    wmat_d = nc.dram_tensor("wmat", [P, 6 * D + 32], F32, kind="ExternalInput")
    wmat1b_d = nc.dram_tensor("wmat1b", [P, 136], BF, kind="ExternalInput")
    brow_d = nc.dram_tensor("brow", [1, 12 * P], F32, kind="ExternalInput")
    encw1T_d = nc.dram_tensor("encw1T", [4, 16], BF, kind="ExternalInput")
    encb1_d = nc.dram_tensor("encb1", [16, 1], F32, kind="ExternalInput")
    w2mT_d = nc.dram_tensor("w2mT", [16, 16], BF, kind="ExternalInput")
    b2m_d = nc.dram_tensor("b2m", [16, 1], F32, kind="ExternalInput")
    encw2T_d = nc.dram_tensor("encw2T", [16, 8], F32, kind="ExternalInput")
    mstT_d = nc.dram_tensor("mstT", [8, 16], F32, kind="ExternalInput")
    corr2_d = nc.dram_tensor("corr2", [8, 1], F32, kind="ExternalInput")

    out_d = nc.dram_tensor("out", [NLP, D], F32, kind="ExternalOutput")

    ae12_d = nc.dram_tensor("ae12", [cfg.EPAD, 16], BF)
    msg1_d = nc.dram_tensor("msg1", [cfg.EPAD, 136], BF)
    cc2_in = nc.dram_tensor("cc2_in", [NLP, 136], BF)
    cc2_out = nc.dram_tensor("cc2_out", [NC * NLP, 136], BF, addr_space="Shared")
    ccR_in = nc.dram_tensor("ccR_in", [8, 1], F32)
    ccR_out = nc.dram_tensor("ccR_out", [8, 1], F32, addr_space="Shared")
    scr16 = nc.dram_tensor("scr16", [1, 16], F32)

    rg = [list(range(NC))]

    with tile.TileContext(nc) as tc, nc.allow_low_precision(reason="bf16 edge pipeline"):
        with (
            tc.tile_pool(name="slab", bufs=1) as slab,
            tc.tile_pool(name="work", bufs=5) as work,
            tc.tile_pool(name="ps", bufs=2, space="PSUM") as ps,
            tc.tile_pool(name="psacc", bufs=2, space="PSUM") as psacc,
        ):
            # ------------- static tiles
            xT_s = slab.tile([P, NLP], F32, tag="xT")
            srcrow_s = slab.tile([P, NE_T], I32, tag="srcrow")
            srcx_s = slab.tile([P, NE_T], I32, tag="srcx")
            xnm_s = slab.tile([P, NBLK * 132], BF, tag="xnm")
            dstrel_s = slab.tile([P, NE_T], BF, tag="dstrel")
            wmat_s = slab.tile([P, 6 * D + 32], F32, tag="wmat")
            wmat1b_s = slab.tile([P, 136], BF, tag="wmat1b")
            nc.sync.dma_start(out=wmat1b_s[:], in_=wmat1b_d.ap())
            nc.sync.dma_start(out=xT_s[:], in_=xT_d.ap())
            nc.sync.dma_start(out=srcrow_s[:], in_=srcrow_d.ap())
            nc.sync.dma_start(out=srcx_s[:], in_=srcx_d.ap())
            nc.sync.dma_start(out=xnm_s[:], in_=xnm_d.ap())
            nc.sync.dma_start(out=dstrel_s[:], in_=dstrel_d.ap())
            nc.sync.dma_start(out=wmat_s[:], in_=wmat_d.ap())

            brow_names = ["bias1", "bias2", "g1", "bn1", "g2", "bn2", "jkb",
                          "asr1", "adt1", "asr2", "adt2", "inb"]
            brc = {}
            for i, nm in enumerate(brow_names):
                t = slab.tile([P, P], F32, tag=f"brc_{nm}")
                nc.sync.dma_start(
                    out=t[:], in_=brow_d.ap()[:, i * P:(i + 1) * P].to_broadcast([P, P]))
                brc[nm] = t

            encw1T_s = slab.tile([4, 16], BF, tag="encw1T")
            encb1_s = slab.tile([16, 1], F32, tag="encb1")
            w2mT_s = slab.tile([16, 16], BF, tag="w2mT")
            b2m_s = slab.tile([16, 1], F32, tag="b2m")
            encw2T_s = slab.tile([16, 8], F32, tag="encw2T")
            mstT_s = slab.tile([8, 16], F32, tag="mstT")
            corr2_s = slab.tile([8, 1], F32, tag="corr2")
            for t, d in [(encw1T_s, encw1T_d), (encb1_s, encb1_d),
                         (w2mT_s, w2mT_d), (b2m_s, b2m_d), (encw2T_s, encw2T_d),
                         (mstT_s, mstT_d), (corr2_s, corr2_d)]:
                nc.sync.dma_start(out=t[:], in_=d.ap())

            ident = slab.tile([P, P], F32, tag="ident")
            make_identity(nc, ident[:])
            identb = slab.tile([P, P], BF, tag="identb")
            nc.vector.tensor_copy(out=identb[:], in_=ident[:])
            iota2_b = slab.tile([P, 2 * P], BF, tag="iota2b")
            iota_f = slab.tile([P, P], F32, tag="iotaf")
            iota_b = slab.tile([P, P], BF, tag="iotab")
            iota_i = slab.tile([P, P], I32, tag="iotai")
            nc.gpsimd.iota(iota_i[:], pattern=[[1, P]], base=0, channel_multiplier=0)
            nc.vector.tensor_copy(out=iota_f[:], in_=iota_i[:])
            nc.vector.tensor_copy(out=iota_b[:], in_=iota_i[:])
            nc.vector.tensor_copy(out=iota2_b[:, 0:P], in_=iota_b[:])
            nc.vector.tensor_copy(out=iota2_b[:, P:2 * P], in_=iota_b[:])
            eps_s = slab.tile([P, 1], F32, tag="eps")
            nc.vector.memset(eps_s[:], LN_EPS)
            iotaP_i = slab.tile([P, 1], I32, tag="iotaPi")
            nc.gpsimd.iota(iotaP_i[:], pattern=[[0, 1]], base=0,
                           channel_multiplier=1)
            iotaP_b = slab.tile([P, 1], BF, tag="iotaPb")
            nc.vector.tensor_copy(out=iotaP_b[:], in_=iotaP_i[:])

            def make_stq(t0, ntile):
                """Transposed one-hot S^T[q, e] for ntile tiles at t0, built
                from a partition-broadcast of dstrelT (no PE transpose)."""
                stqb = work.tile([P, 2 * P], BF, tag="estqb")
                nc.sync.dma_start(
                    out=stqb[:, 0:ntile * P],
                    in_=dstrelT_d.ap()[:, t0 * P:(t0 + ntile) * P]
                        .to_broadcast([P, ntile * P]))
                stq = work.tile([P, 2 * P], BF, tag="estq")
                nc.vector.tensor_tensor(
                    out=stq[:, 0:ntile * P],
                    in0=iotaP_b[:].to_broadcast([P, ntile * P]),
                    in1=stqb[:, 0:ntile * P],
                    op=mybir.AluOpType.is_equal)
                return stq

            xs1_s = slab.tile([P, NLP], BF, tag="xs1")
            xs2_s = slab.tile([P, NLP], BF, tag="xs2")
            h0_s = slab.tile([P, NLP], F32, tag="h0")
            h1_s = slab.tile([P, NLP], F32, tag="h1")
            h1T_s = slab.tile([P, NLP], F32, tag="h1T")
            asad1_s = slab.tile([P, NBLK * 16], BF, tag="asad1")
            asad2_s = slab.tile([P, NBLK * 16], BF, tag="asad2")
            eaacc_s = slab.tile([16, max(NE_T // 2, 1)], F32, tag="eaacc")
            aself_s = slab.tile([P, 16], F32, tag="aself")

            # ------------- phase A: node projections layer 1 (local only)
            for b in range(NBLK):
                nsl = slice(b * P, (b + 1) * P)
                lhs = xT_s[:, nsl]
                p0 = ps.tile([P, P], F32, tag="pp")
                nc.tensor.matmul(p0[:], lhsT=lhs, rhs=wmat_s[:, 0:D],
                                 start=True, stop=True)
                nc.vector.tensor_add(out=h0_s[:, nsl], in0=p0[:], in1=brc["inb"][:])
                p1 = ps.tile([P, P], F32, tag="pp")
                nc.tensor.matmul(p1[:], lhsT=lhs, rhs=wmat_s[:, D:2 * D],
                                 start=True, stop=True)
                nc.vector.tensor_copy(out=xs1_s[:, nsl], in_=p1[:])
                pa = ps.tile([P, 16], F32, tag="pp")
                nc.tensor.matmul(pa[:], lhsT=lhs, rhs=wmat_s[:, 768:784],
                                 start=True, stop=True)
                nc.vector.tensor_copy(out=asad1_s[:, b * 16:(b + 1) * 16],
                                      in_=pa[:])

            # ------------- phase B: edge features + a_edge for both layers
            for tp in range(NE_T // 2):
                t0 = 2 * tp
                Sf2 = make_stq(t0, 2)
                efTp2 = ps.tile([4, 2 * P], BF, tag="ppb")
                for k in range(2):
                    t = t0 + k
                    b = BLK_OF[t]
                    gs = work.tile([P, 132], BF, tag="fgs")
                    nc.gpsimd.indirect_dma_start(
                        out=gs[:], out_offset=None, in_=xaug_d.ap(),
                        in_offset=bass.IndirectOffsetOnAxis(ap=srcx_s[:, t:t + 1], axis=0))
                    # stage [xs1|asrc1][src] for layer 1 (read back by plain DMA)
                    gsTp = ps.tile([P, P], BF, tag="ppb")
                    nc.tensor.transpose(out=gsTp[:], in_=gs[:, 0:P],
                                        identity=identb[:])
                    gsT = work.tile([P, P], BF, tag="fgsT")
                    nc.vector.tensor_copy(out=gsT[:], in_=gsTp[:])
                    stg = ps.tile([P, 136], F32, tag="pp")
                    nc.tensor.matmul(stg[:], lhsT=gsT[:], rhs=wmat1b_s[:],
                                     start=True, stop=True)
                    stgb = work.tile([P, 136], BF, tag="fstgb")
                    nc.scalar.activation(
                        out=stgb[:], in_=stg[:],
                        func=mybir.ActivationFunctionType.Identity)
                    nc.sync.dma_start(out=msg1_d.ap()[t * P:(t + 1) * P, :],
                                      in_=stgb[:])
                    gdp = ps.tile([P, 132], F32, tag="pp")
                    nc.tensor.matmul(gdp[:], lhsT=Sf2[:, k * P:(k + 1) * P],
                                     rhs=xnm_s[:, b * 132:(b + 1) * 132],
                                     start=True, stop=True)
                    prod = work.tile([P, P], F32, tag="fprod", bufs=3)
                    nc.vector.tensor_mul(out=prod[:], in0=gs[:, 0:P], in1=gdp[:, 0:P])
                    ef = work.tile([P, 4], BF, tag="fef")
                    nc.vector.reduce_sum(out=ef[:, 0:1], in_=prod[:],
                                         axis=mybir.AxisListType.X)
                    nn = work.tile([P, 2], F32, tag="fnn")
                    nc.vector.tensor_add(out=nn[:, 0:1], in0=gs[:, 128:129],
                                         in1=gdp[:, 128:129])
                    nc.vector.tensor_scalar(out=nn[:, 1:2], in0=ef[:, 0:1],
                                            scalar1=-2.0, scalar2=None,
                                            op0=mybir.AluOpType.mult)
                    nc.vector.tensor_add(out=ef[:, 1:2], in0=nn[:, 0:1], in1=nn[:, 1:2])
                    nc.vector.tensor_copy(out=ef[:, 2:3], in_=gs[:, 128:129])
                    nc.vector.tensor_copy(out=ef[:, 3:4], in_=gdp[:, 128:129])
                    nc.tensor.transpose(out=efTp2[:, k * P:(k + 1) * P], in_=ef[:],
                                        identity=identb[:])
                efT2 = work.tile([4, 2 * P], BF, tag="fefT")
                nc.vector.tensor_copy(out=efT2[:], in_=efTp2[:])
                h1ep = ps.tile([16, 2 * P], F32, tag="pp")
                nc.tensor.matmul(h1ep[:], lhsT=encw1T_s[:], rhs=efT2[:],
                                 start=True, stop=True)
                h1e = work.tile([16, 2 * P], BF, tag="fh1e")
                nc.scalar.activation(out=h1e[:], in_=h1ep[:],
                                     func=mybir.ActivationFunctionType.Relu,
                                     bias=encb1_s[:],
                                     accum_out=eaacc_s[:, tp:tp + 1])
                aep = ps.tile([16, 2 * P], F32, tag="pp")
                nc.tensor.matmul(aep[:], lhsT=w2mT_s[:], rhs=h1e[:],
                                 start=True, stop=True)
                aeT = work.tile([16, 2 * P], BF, tag="faeT")
                nc.scalar.activation(out=aeT[:], in_=aep[:],
                                     func=mybir.ActivationFunctionType.Identity,
                                     bias=b2m_s[:])
                aeRp = ps.tile([P, 32], BF, tag="ppb2")
                for k in range(2):
                    nc.tensor.transpose(out=aeRp[:, k * 16:(k + 1) * 16],
                                        in_=aeT[:, k * P:(k + 1) * P],
                                        identity=identb[:16, :16])
                aeR = work.tile([P, 32], BF, tag="faeR")
                nc.vector.tensor_copy(out=aeR[:], in_=aeRp[:])
                nc.sync.dma_start(
                    out=ae12_d.ap()[t0 * P:(t0 + 2) * P, :]
                        .rearrange("(a p) j -> p a j", p=P),
                    in_=aeR[:].rearrange("p (a j) -> p a j", a=2))

            # sum of ea over real edges -> AllReduce -> a_self for both layers
            sr = work.tile([16, 1], F32, tag="sr")
            nc.vector.reduce_sum(out=sr[:], in_=eaacc_s[:],
                                 axis=mybir.AxisListType.X)
            eap = ps.tile([8, 1], F32, tag="pp")
            nc.tensor.matmul(eap[:], lhsT=encw2T_s[:], rhs=sr[:],
                             start=True, stop=True)
            eas = work.tile([8, 1], F32, tag="eas")
            nc.vector.tensor_add(out=eas[:], in0=eap[:], in1=corr2_s[:])
            nc.sync.dma_start(out=ccR_in.ap(), in_=eas[:])
            nc.gpsimd.collective_compute(
                "AllReduce", mybir.AluOpType.add, replica_groups=rg,
                ins=[ccR_in.ap().opt()], outs=[ccR_out.ap().opt()])
            gsum = work.tile([8, 1], F32, tag="gsum")
            nc.sync.dma_start(out=gsum[:], in_=ccR_out.ap())
            mean8 = work.tile([8, 1], F32, tag="mean8")
            nc.vector.tensor_scalar(out=mean8[:], in0=gsum[:],
                                    scalar1=1.0 / cfg.E, scalar2=None,
                                    op0=mybir.AluOpType.mult)
            asp = ps.tile([16, 1], F32, tag="pp")
            nc.tensor.matmul(asp[:], lhsT=mstT_s[:], rhs=mean8[:],
                             start=True, stop=True)
            c16 = work.tile([16, 1], F32, tag="c16")
            nc.vector.tensor_copy(out=c16[:], in_=asp[:])
            r16p = ps.tile([1, 16], F32, tag="pp")
            nc.tensor.transpose(out=r16p[:], in_=c16[:], identity=ident[:16, :16])
            r16 = work.tile([1, 16], F32, tag="r16")
            nc.vector.tensor_copy(out=r16[:], in_=r16p[:])
            nc.sync.dma_start(out=scr16.ap(), in_=r16[:])
            nc.sync.dma_start(out=aself_s[:], in_=scr16.ap().to_broadcast([P, 16]))

            # ------------- GAT layers
            def gat_layer(lyr, cco, xs_s, asad_s, hres_s, hpre_s):
                """lyr: 0 or 1. hres_s: residual input (h0 or h1).
                hpre_s: where pre-LN output rows go (LN applied in a later
                batched pass). Layer 0 loads staged per-edge messages from
                msg1_d with plain DMAs; layer 1 gathers from cc2_out."""
                ao = lyr * 8
                gdt = BF
                gbias = brc["bias1"] if lyr == 0 else brc["bias2"]
                for b in range(NBLK):
                    nsl = slice(b * P, (b + 1) * P)
                    asl = slice(b * 16, b * 16 + 8)
                    dsl = slice(b * 16 + 8, b * 16 + 16)
                    acc = psacc.tile([P, 136], F32, tag="acc")
                    for tp2 in range(TB[b] // 2 + (TB[b] % 2)):
                        tt0 = 2 * tp2
                        npair = 2 if tt0 + 1 < TB[b] else 1
                        t0 = OFF[b] + tt0
                        gxs = work.tile([P, 2 * 136], gdt, tag="egxs", bufs=8)
                        if lyr == 0:
                            nc.sync.dma_start(
                                out=gxs[:, 0:npair * 136]
                                    .rearrange("p (a j) -> p a j", j=136),
                                in_=msg1_d.ap()[t0 * P:(t0 + npair) * P, :]
                                    .rearrange("(a p) j -> p a j", p=P))
                        else:
                            for k in range(npair):
                                nc.gpsimd.indirect_dma_start(
                                    out=gxs[:, k * 136:(k + 1) * 136],
                                    out_offset=None,
                                    in_=cco.ap(),
                                    in_offset=bass.IndirectOffsetOnAxis(
                                        ap=srcrow_s[:, t0 + k:t0 + k + 1], axis=0))
                        aeR = work.tile([P, 32], BF, tag="eaeR")
                        nc.sync.dma_start(
                            out=aeR[:, 0:npair * 16].rearrange("p (a j) -> p a j", j=16),
                            in_=ae12_d.ap()[t0 * P:(t0 + npair) * P, :]
                                .rearrange("(a p) j -> p a j", p=P))
                        S2 = work.tile([P, 2 * P], BF, tag="gS2")
                        nc.vector.tensor_tensor(
                            out=S2[:, 0:npair * P].rearrange("p (a q) -> p a q", q=P),
                            in0=dstrel_s[:, t0:t0 + npair]
                                .rearrange("p (a o) -> p a o", o=1)
                                .to_broadcast([P, npair, P]),
                            in1=iota2_b[:, 0:npair * P].rearrange("p (a q) -> p a q", q=P),
                            op=mybir.AluOpType.is_equal)
                        stqg = make_stq(t0, npair)
                        adp = ps.tile([P, 16], F32, tag="ppb2")
                        for k in range(npair):
                            nc.tensor.matmul(adp[:, k * 8:(k + 1) * 8],
                                             lhsT=stqg[:, k * P:(k + 1) * P],
                                             rhs=asad_s[:, dsl],
                                             start=True, stop=True)
                        al = work.tile([P, 16], F32, tag="eal")
                        nc.vector.tensor_add(
                            out=al[:, 0:npair * 8].rearrange("p (a q) -> p a q", q=8),
                            in0=gxs[:].rearrange("p (a q) -> p a q", q=136)[
                                :, 0:npair, D:136],
                            in1=adp[:, 0:npair * 8].rearrange("p (a q) -> p a q", q=8))
                        nc.vector.tensor_add(
                            out=al[:, 0:npair * 8].rearrange("p (a q) -> p a q", q=8),
                            in0=al[:, 0:npair * 8].rearrange("p (a q) -> p a q", q=8),
                            in1=aeR[:, 0:npair * 16].rearrange("p (a j) -> p a j", j=16)[
                                :, :, ao:ao + 8])
                        al2 = work.tile([P, 16], F32, tag="eal2")
                        nc.vector.tensor_scalar(out=al2[:, 0:npair * 8],
                                                in0=al[:, 0:npair * 8], scalar1=LRELU,
                                                scalar2=None, op0=mybir.AluOpType.mult)
                        nc.vector.tensor_tensor(out=al[:, 0:npair * 8],
                                                in0=al[:, 0:npair * 8],
                                                in1=al2[:, 0:npair * 8],
                                                op=mybir.AluOpType.max)
                        msg = work.tile([P, 2 * 136], BF, tag="emsg")
                        nc.scalar.activation(
                            out=msg[:].rearrange("p (a q) -> p a q", q=136)[
                                :, 0:npair, D:136],
                            in_=al[:, 0:npair * 8].rearrange("p (a q) -> p a q", q=8),
                            func=mybir.ActivationFunctionType.Exp)
                        nc.vector.tensor_tensor(
                            out=msg[:].rearrange("p (a q) -> p a q", q=136)[
                                :, 0:npair, 0:D].rearrange(
                                "p a (hh cc) -> p a hh cc", hh=H),
                            in0=gxs[:].rearrange("p (a q) -> p a q", q=136)[
                                :, 0:npair, 0:D].rearrange(
                                "p a (hh cc) -> p a hh cc", hh=H),
                            in1=msg[:].rearrange("p (a q) -> p a q", q=136)[
                                :, 0:npair, D:136].rearrange(
                                "p a (hh o) -> p a hh o", o=1)
                                .to_broadcast([P, npair, H, C]),
                            op=mybir.AluOpType.mult)
                        for k in range(npair):
                            nc.tensor.matmul(
                                acc[:], lhsT=S2[:, k * P:(k + 1) * P],
                                rhs=msg[:, k * 136:(k + 1) * 136],
                                start=(tt0 + k == 0), stop=(tt0 + k == TB[b] - 1))
                    # ---- block flush: self loop + normalize + elu + residual
                    als = work.tile([P, 8], F32, tag="fal")
                    nc.vector.tensor_add(out=als[:], in0=asad_s[:, asl],
                                         in1=asad_s[:, dsl])
                    nc.vector.tensor_add(out=als[:], in0=als[:],
                                         in1=aself_s[:, ao:ao + 8])
                    als2 = work.tile([P, 8], F32, tag="fal2")
                    nc.vector.tensor_scalar(out=als2[:], in0=als[:], scalar1=LRELU,
                                            scalar2=None, op0=mybir.AluOpType.mult)
                    nc.vector.tensor_tensor(out=als[:], in0=als[:], in1=als2[:],
                                            op=mybir.AluOpType.max)
                    exps = work.tile([P, 8], F32, tag="fexp")
                    nc.scalar.activation(out=exps[:], in_=als[:],
                                         func=mybir.ActivationFunctionType.Exp)
                    den = work.tile([P, 8], F32, tag="fden")
                    nc.vector.tensor_add(out=den[:], in0=acc[:, D:136], in1=exps[:])
                    nc.vector.reciprocal(out=den[:], in_=den[:])
                    smsg = work.tile([P, P], F32, tag="fsmsg")
                    nc.vector.tensor_tensor(
                        out=smsg[:].rearrange("p (h c) -> p h c", h=H),
                        in0=xs_s[:, nsl].rearrange("p (h c) -> p h c", h=H),
                        in1=exps[:].rearrange("p (h o) -> p h o", o=1)
                            .to_broadcast([P, H, C]),
                        op=mybir.AluOpType.mult)
                    num = work.tile([P, P], F32, tag="fnum")
                    nc.vector.tensor_add(out=num[:], in0=acc[:, 0:D], in1=smsg[:])
                    nc.vector.tensor_tensor(
                        out=num[:].rearrange("p (h c) -> p h c", h=H),
                        in0=num[:].rearrange("p (h c) -> p h c", h=H),
                        in1=den[:].rearrange("p (h o) -> p h o", o=1)
                            .to_broadcast([P, H, C]),
                        op=mybir.AluOpType.mult)
                    nc.vector.tensor_add(out=num[:], in0=num[:], in1=gbias[:])
                    # elu(x) = max(x,0) + (exp(min(x,0)) - 1)
                    mn = work.tile([P, P], F32, tag="fmn")
                    nc.vector.tensor_scalar(out=mn[:], in0=num[:], scalar1=0.0,
                                            scalar2=None, op0=mybir.AluOpType.min)
                    nc.scalar.activation(out=mn[:], in_=mn[:],
                                         func=mybir.ActivationFunctionType.Exp)
                    nc.vector.tensor_scalar(out=num[:], in0=num[:], scalar1=0.0,
                                            scalar2=None, op0=mybir.AluOpType.max)
                    nc.vector.tensor_add(out=num[:], in0=num[:], in1=mn[:])
                    nc.vector.tensor_scalar(out=num[:], in0=num[:], scalar1=-1.0,
                                            scalar2=None, op0=mybir.AluOpType.add)
                    # residual; LN deferred to a batched pass
                    nc.vector.tensor_add(out=hpre_s[:, nsl], in0=num[:],
                                         in1=hres_s[:, nsl])

            def ln_block(src_ap, gg, gb):
                """LayerNorm src_ap in place; returns normalized tile view."""
                st6 = work.tile([P, 6], F32, tag="fst6")
                nc.vector.bn_stats(out=st6[:], in_=src_ap)
                mv = work.tile([P, 2], F32, tag="fmv")
                nc.vector.bn_aggr(out=mv[:], in_=st6[:])
                sd = work.tile([P, 1], F32, tag="fsd")
                nc.scalar.activation(out=sd[:], in_=mv[:, 1:2],
                                     func=mybir.ActivationFunctionType.Sqrt,
                                     bias=eps_s[:])
                nc.vector.reciprocal(out=sd[:], in_=sd[:])
                nc.vector.tensor_scalar(out=src_ap, in0=src_ap,
                                        scalar1=mv[:, 0:1], scalar2=sd[:],
                                        op0=mybir.AluOpType.subtract,
                                        op1=mybir.AluOpType.mult)
                nc.vector.tensor_mul(out=src_ap, in0=src_ap, in1=gg[:])
                nc.vector.tensor_add(out=src_ap, in0=src_ap, in1=gb[:])

            gat_layer(0, None, xs1_s, asad1_s, h0_s, h1_s)

            # ------------- LN pass layer 1 (batched: one act-table swap)
            for b in range(NBLK):
                nsl = slice(b * P, (b + 1) * P)
                ln_block(h1_s[:, nsl], brc["g1"], brc["bn1"])
                h1Tp = ps.tile([P, P], F32, tag="pp")
                nc.tensor.transpose(out=h1Tp[:], in_=h1_s[:, nsl],
                                    identity=ident[:])
                nc.vector.tensor_copy(out=h1T_s[:, nsl], in_=h1Tp[:])

            # ------------- phase D: node projections layer 2
            for b in range(NBLK):
                nsl = slice(b * P, (b + 1) * P)
                p2 = ps.tile([P, P], F32, tag="pp")
                nc.tensor.matmul(p2[:], lhsT=h1T_s[:, nsl], rhs=wmat_s[:, 2 * D:3 * D],
                                 start=True, stop=True)
                nc.vector.tensor_copy(out=xs2_s[:, nsl], in_=p2[:])
                pa2 = ps.tile([P, 16], F32, tag="pp")
                nc.tensor.matmul(pa2[:], lhsT=h1T_s[:, nsl],
                                 rhs=wmat_s[:, 784:800], start=True, stop=True)
                nc.vector.tensor_copy(out=asad2_s[:, b * 16:(b + 1) * 16],
                                      in_=pa2[:])
                nc.sync.dma_start(out=cc2_in.ap()[nsl, 0:D], in_=xs2_s[:, nsl])
                nc.sync.dma_start(out=cc2_in.ap()[nsl, D:136],
                                  in_=asad2_s[:, b * 16:b * 16 + 8])

            nc.gpsimd.collective_compute(
                "AllGather", mybir.AluOpType.bypass, replica_groups=rg,
                ins=[cc2_in.ap().opt()], outs=[cc2_out.ap().opt()])

            # layer 2 writes pre-LN h2 into xT_s (no longer needed)
            gat_layer(1, cc2_out, xs2_s, asad2_s, h1_s, xT_s)

            # ------------- LN2 + JK + output (batched act tables)
            for b in range(NBLK):
                nsl = slice(b * P, (b + 1) * P)
                ln_block(xT_s[:, nsl], brc["g2"], brc["bn2"])
                h2Tp = ps.tile([P, P], F32, tag="pp")
                nc.tensor.transpose(out=h2Tp[:], in_=xT_s[:, nsl],
                                    identity=ident[:])
                h2T = work.tile([P, P], F32, tag="fh2T")
                nc.vector.tensor_copy(out=h2T[:], in_=h2Tp[:])
                h0Tp = ps.tile([P, P], F32, tag="pp")
                nc.tensor.transpose(out=h0Tp[:], in_=h0_s[:, nsl],
                                    identity=ident[:])
                h0T = work.tile([P, P], F32, tag="fh0T")
                nc.vector.tensor_copy(out=h0T[:], in_=h0Tp[:])
                jkp = psacc.tile([P, P], F32, tag="acc")
                nc.tensor.matmul(jkp[:], lhsT=h0T[:],
                                 rhs=wmat_s[:, 3 * D:4 * D],
                                 start=True, stop=False)
                nc.tensor.matmul(jkp[:], lhsT=h1T_s[:, nsl],
                                 rhs=wmat_s[:, 4 * D:5 * D],
                                 start=False, stop=False)
                nc.tensor.matmul(jkp[:], lhsT=h2T[:],
                                 rhs=wmat_s[:, 5 * D:6 * D],
                                 start=False, stop=True)
                outt = work.tile([P, P], F32, tag="foutt")
                nc.vector.tensor_add(out=outt[:], in0=jkp[:], in1=brc["jkb"][:])
                nc.sync.dma_start(out=out_d.ap()[nsl, :], in_=outt[:])

    nc.compile()
    return nc


# ---------------------------------------------------------------- entry

FULL_CFG = dict(N=50000, E=600000, NC=8, H=8, C=16)


def kernel(**inputs):
    cfg = Cfg(**FULL_CFG)
    in_maps = host_prep(cfg, inputs)
    nc = build_program(cfg)
    res = bass_utils.run_bass_kernel_spmd(
        nc, in_maps, core_ids=list(range(cfg.NC)))
    out = np.concatenate(
        [res.results[c]["out"][:cfg.NLOC] for c in range(cfg.NC)], axis=0)
    return out.astype(np.float32)

